# revision 11
# baseline (speedup 1.0000x reference)
"""Distributed CLIP loss on 8 TRN2 NeuronCores.

Contract: kernel(**inputs) takes the FULL inputs
  image_features (8192, 1024) f32, text_features (8192, 1024) f32,
  logit_scale () f32
and returns the FULL output: scalar f32 loss.

Strategy (data parallel over batch):
  - shard image rows 8 ways (1024 rows/core); every core gets all text rows
  - core computes L^T tile-block = logit_scale * text_tile @ I_c^T via
    TensorE (bf16, fp32 PSUM accumulation)
  - E = exp(L^T - C) on ScalarE; its accum_out gives the text-direction
    (t2i) partial column sums for free; VectorE accumulates A += E for the
    image-direction (i2t) row sums
  - diagonal logits computed as elementwise product + reduce (exact rows
    of text matching this core's image rows are passed as an extra input)
  - one 33 KB AllReduce combines the t2i partial sums + per-core scalars;
    every core finishes the identical final scalar
Host side only reshapes/casts inputs (layout prep) and reads back core 0's
scalar.
"""

import numpy as np
import ml_dtypes

import concourse.bass as bass
import concourse.tile as tile
from concourse import bacc, mybir
from concourse.bass_utils import run_bass_kernel_spmd

N = 8192
D = 1024
CORES = 8
LOCAL = N // CORES          # image rows per core
KC = D // 128               # contraction chunks of 128
TT = N // 128               # text tiles of 128 rows
C_SHIFT = 25.0              # exp shift: logits for this data are within ~[-20, 20]

BF16 = mybir.dt.bfloat16
F32 = mybir.dt.float32
AF = mybir.ActivationFunctionType
ALU = mybir.AluOpType

_CACHE = {}


def _build_bass():
    from contextlib import ExitStack

    nc = bacc.Bacc(None, num_devices=CORES)

    # it: scaled image shard, transposed:   it[p, kc, i] = s * I_c[i, kc*128+p]
    # tt: all text rows, tiled+transposed:  tt[t, p, kc, c] = T[t*128+c, kc*128+p]
    # td: text shard matching local images: td[p, kc, i] = T_c[i, kc*128+p]
    it_d = nc.declare_dram_parameter("it", [128, KC, LOCAL], BF16, isOutput=False)
    tt_d = nc.declare_dram_parameter("tt", [TT, 128, KC, 128], BF16, isOutput=False)
    td_d = nc.declare_dram_parameter("td", [128, KC, LOCAL], BF16, isOutput=False)
    out_d = nc.declare_dram_parameter("out", [1, 1], F32, isOutput=True)

    with tile.TileContext(nc) as tc, ExitStack() as ctx:
        singles = ctx.enter_context(tc.tile_pool(name="singles", bufs=1))
        tpool = ctx.enter_context(tc.tile_pool(name="tpool", bufs=4))
        epool = ctx.enter_context(tc.tile_pool(name="epool", bufs=4))
        ppool = ctx.enter_context(tc.tile_pool(name="ppool", bufs=2, space="PSUM"))
        tailp = ctx.enter_context(tc.tile_pool(name="tailp", bufs=2, space="PSUM"))
        drp = ctx.enter_context(tc.tile_pool(name="drp", bufs=1, space="DRAM"))

        IT = singles.tile([128, KC, LOCAL], BF16)
        nc.sync.dma_start(out=IT, in_=it_d[:])
        TD = singles.tile([128, KC, LOCAL], BF16)
        nc.sync.dma_start(out=TD, in_=td_d[:])

        A = singles.tile([128, LOCAL], F32)        # i2t partial sums by text-partition
        nc.vector.memset(A, 0.0)
        colacc = singles.tile([128, TT], F32)      # t2i partial sums, [p, tt]
        ones = singles.tile([128, 1], F32)
        nc.vector.memset(ones, 1.0)
        nshift = singles.tile([128, 1], F32)
        nc.vector.memset(nshift, -C_SHIFT)

        # ---- main loop: 64 text tiles x (16 matmuls + exp + accumulate) ----
        for t in range(TT):
            ttile = tpool.tile([128, KC, 128], BF16)
            nc.sync.dma_start(out=ttile, in_=tt_d[t])
            psum = ppool.tile([128, LOCAL], F32)
            for h in range(2):
                for kc in range(KC):
                    nc.tensor.matmul(
                        psum[:, h * 512:(h + 1) * 512],
                        ttile[:, kc, :],
                        IT[:, kc, h * 512:(h + 1) * 512],
                        start=(kc == 0),
                        stop=(kc == KC - 1),
                    )
            E = epool.tile([128, LOCAL], F32)
            nc.scalar.activation(
                E, psum, AF.Exp, bias=nshift, scale=1.0,
                accum_out=colacc[:, t:t + 1],
            )
            nc.vector.tensor_add(A, A, E)

        # ---- diagonal: sum_i s*<I_c[i], T_c[i]> via elementwise mul+reduce ----
        # TensorScalarPtr has a single sync-wait slot; advance the DVE clock
        # past both input DMAs with tiny reads first so the big op needs none.
        dwarm = singles.tile([1, 2], F32)
        nc.vector.tensor_copy(dwarm[:, 0:1], IT[0:1, 0, 0:1])
        nc.vector.tensor_copy(dwarm[:, 1:2], TD[0:1, 0, 0:1])
        dprod = singles.tile([128, KC, LOCAL], F32)
        dacc = singles.tile([128, 1], F32)
        nc.vector.scalar_tensor_tensor(
            out=dprod, in0=IT, scalar=1.0, in1=TD,
            op0=ALU.mult, op1=ALU.mult, accum_out=dacc,
        )

        # ---- i2t row sums: partition-reduce A via ones-matmul, then ln ----
        psum_r = tailp.tile([1, LOCAL], F32, tag="tail")
        for h in range(2):
            nc.tensor.matmul(
                psum_r[:, h * 512:(h + 1) * 512], ones, A[:, h * 512:(h + 1) * 512],
                start=True, stop=True,
            )
        lse_r = singles.tile([1, LOCAL], F32)
        s_row = singles.tile([1, 1], F32)
        nc.scalar.activation(lse_r, psum_r, AF.Ln, accum_out=s_row)

        # ---- diag total: partition-reduce dacc ----
        psum_d = tailp.tile([1, LOCAL], F32, tag="tail")
        nc.tensor.matmul(psum_d[:, 0:1], ones, dacc, start=True, stop=True)

        # ---- AllReduce: [p, 0:64] t2i partials, [0, 64] s_row, [0, 65] diag ----
        arin = singles.tile([128, 66], F32)
        nc.vector.memset(arin, 0.0)
        nc.vector.tensor_copy(arin[:, 0:TT], colacc)
        nc.vector.tensor_copy(arin[0:1, TT:TT + 1], s_row)
        nc.vector.tensor_copy(arin[0:1, TT + 1:TT + 2], psum_d[0:1, 0:1])

        cc_in = drp.tile([128, 66], F32)
        cc_out = drp.tile([128, 66], F32, addr_space="Shared")
        nc.sync.dma_start(out=cc_in, in_=arin)
        nc.gpsimd.collective_compute(
            "AllReduce",
            ALU.add,
            replica_groups=[list(range(CORES))],
            ins=[cc_in[:]],
            outs=[cc_out[:]],
        )
        arout = singles.tile([128, 66], F32)
        nc.sync.dma_start(out=arout, in_=cc_out)

        # ---- t2i logsumexp over the reduced column sums ----
        lse_c = singles.tile([128, TT], F32)
        cl = singles.tile([128, 1], F32)
        nc.scalar.activation(lse_c, arout[:, 0:TT], AF.Ln, accum_out=cl)
        psum_s = tailp.tile([1, LOCAL], F32, tag="tail")
        nc.tensor.matmul(psum_s[:, 0:1], ones, cl, start=True, stop=True)

        # ---- loss = (S_col + S_row - 2*diag_tot)/(2N) + C ----
        f0 = singles.tile([1, 1], F32)
        nc.vector.tensor_add(f0, psum_s[0:1, 0:1], arout[0:1, TT:TT + 1])
        f1 = singles.tile([1, 1], F32)
        nc.vector.scalar_tensor_tensor(
            out=f1, in0=arout[0:1, TT + 1:TT + 2], scalar=-2.0, in1=f0,
            op0=ALU.mult, op1=ALU.add,
        )
        res = singles.tile([1, 1], F32)
        nc.scalar.activation(res, f1, AF.Copy, bias=C_SHIFT, scale=1.0 / (2.0 * N))
        nc.sync.dma_start(out=out_d[:], in_=res)

    nc.finalize()
    return nc


def _prep_inputs(image_features, text_features, logit_scale):
    s = float(np.asarray(logit_scale, dtype=np.float32))
    I = np.asarray(image_features, dtype=np.float32)
    T = np.asarray(text_features, dtype=np.float32)
    bf16 = ml_dtypes.bfloat16

    # tt[t, p, kc, c] = T[t*128 + c, kc*128 + p]
    tt = np.ascontiguousarray(
        T.reshape(TT, 128, KC, 128).transpose(0, 3, 2, 1)
    ).astype(bf16)

    in_maps = []
    for c in range(CORES):
        Ic = I[c * LOCAL:(c + 1) * LOCAL] * s
        it = np.ascontiguousarray(
            Ic.reshape(LOCAL, KC, 128).transpose(2, 1, 0)
        ).astype(bf16)
        Tc = T[c * LOCAL:(c + 1) * LOCAL]
        td = np.ascontiguousarray(
            Tc.reshape(LOCAL, KC, 128).transpose(2, 1, 0)
        ).astype(bf16)
        in_maps.append({"it": it, "tt": tt, "td": td})
    return in_maps


def _get_nc():
    if "nc" not in _CACHE:
        _CACHE["nc"] = _build_bass()
    return _CACHE["nc"]


def kernel(image_features, text_features, logit_scale, _trace=False):
    nc = _get_nc()
    in_maps = _prep_inputs(image_features, text_features, logit_scale)
    out = run_bass_kernel_spmd(nc, in_maps, list(range(CORES)), trace=_trace)
    loss = np.float32(out.results[0]["out"][0, 0])
    if _trace:
        return loss, out
    return loss


# revision 12
# speedup vs baseline: 1.0835x; 1.0835x over previous
"""Distributed CLIP loss on 8 TRN2 NeuronCores.

Contract: kernel(**inputs) takes the FULL inputs
  image_features (8192, 1024) f32, text_features (8192, 1024) f32,
  logit_scale () f32
and returns the FULL output: scalar f32 loss.

Strategy (data parallel over batch):
  - shard image rows 8 ways (1024 rows/core); every core gets all text rows
  - core computes L^T tile-block = logit_scale * text_tile @ I_c^T via
    TensorE (bf16, fp32 PSUM accumulation)
  - E = exp(L^T - C) on ScalarE; its accum_out gives the text-direction
    (t2i) partial column sums for free; VectorE accumulates A += E for the
    image-direction (i2t) row sums
  - diagonal logits computed as elementwise product + reduce (exact rows
    of text matching this core's image rows are passed as an extra input)
  - one 33 KB AllReduce combines the t2i partial sums + per-core scalars;
    every core finishes the identical final scalar
Host side only reshapes/casts inputs (layout prep) and reads back core 0's
scalar.
"""

import numpy as np
import ml_dtypes

import concourse.bass as bass
import concourse.tile as tile
from concourse import bacc, mybir
from concourse.bass_utils import run_bass_kernel_spmd

N = 8192
D = 1024
CORES = 8
LOCAL = N // CORES          # image rows per core
KC = D // 128               # contraction chunks of 128
TT = N // 128               # text tiles of 128 rows
C_SHIFT = 25.0              # exp shift: logits for this data are within ~[-20, 20]

BF16 = mybir.dt.bfloat16
F32 = mybir.dt.float32
AF = mybir.ActivationFunctionType
ALU = mybir.AluOpType

_CACHE = {}


def _build_bass():
    from contextlib import ExitStack

    nc = bacc.Bacc(None, num_devices=CORES)

    # it: scaled image shard, transposed:   it[p, kc, i] = s * I_c[i, kc*128+p]
    # tt: all text rows, tiled+transposed:  tt[t, p, kc, c] = T[t*128+c, kc*128+p]
    # td: text shard matching local images: td[p, kc, i] = T_c[i, kc*128+p]
    it_d = nc.declare_dram_parameter("it", [128, KC, LOCAL], BF16, isOutput=False)
    tt_d = nc.declare_dram_parameter("tt", [TT, 128, KC, 128], BF16, isOutput=False)
    td_d = nc.declare_dram_parameter("td", [128, KC, LOCAL], BF16, isOutput=False)
    out_d = nc.declare_dram_parameter("out", [1, 1], F32, isOutput=True)

    with tile.TileContext(nc) as tc, ExitStack() as ctx:
        singles = ctx.enter_context(tc.tile_pool(name="singles", bufs=1))
        tpool = ctx.enter_context(tc.tile_pool(name="tpool", bufs=4))
        epool = ctx.enter_context(tc.tile_pool(name="epool", bufs=4))
        ppool = ctx.enter_context(tc.tile_pool(name="ppool", bufs=2, space="PSUM"))
        tailp = ctx.enter_context(tc.tile_pool(name="tailp", bufs=2, space="PSUM"))
        drp = ctx.enter_context(tc.tile_pool(name="drp", bufs=1, space="DRAM"))

        # Per-kc IT chunks: 8 independent DMAs so the first matmul group can
        # start as soon as its inputs land rather than after one 2 MB DMA.
        IT = singles.tile([128, KC, LOCAL], BF16)
        for kc in range(KC):
            nc.sync.dma_start(out=IT[:, kc, :], in_=it_d[:, kc, :])

        A = singles.tile([128, LOCAL], F32)        # i2t partial sums by text-partition
        nc.vector.memset(A, 0.0)
        colacc = singles.tile([128, TT], F32)      # t2i partial sums, [p, tt]
        ones = singles.tile([128, 1], F32)
        nc.vector.memset(ones, 1.0)
        nshift = singles.tile([128, 1], F32)
        nc.vector.memset(nshift, -C_SHIFT)

        TD = singles.tile([128, KC, LOCAL], BF16)
        dprod = singles.tile([128, KC, LOCAL], F32)
        dacc8 = singles.tile([128, KC], F32)

        # ---- main loop: 64 text tiles x (16 matmuls + exp + accumulate) ----
        for t in range(TT):
            ttile = tpool.tile([128, KC, 128], BF16)
            nc.sync.dma_start(out=ttile, in_=tt_d[t])
            psum = ppool.tile([128, LOCAL], F32)
            for h in range(2):
                for kc in range(KC):
                    nc.tensor.matmul(
                        psum[:, h * 512:(h + 1) * 512],
                        ttile[:, kc, :],
                        IT[:, kc, h * 512:(h + 1) * 512],
                        start=(kc == 0),
                        stop=(kc == KC - 1),
                    )
            E = epool.tile([128, LOCAL], F32)
            nc.scalar.activation(
                E, psum, AF.Exp, bias=nshift, scale=1.0,
                accum_out=colacc[:, t:t + 1],
            )
            nc.vector.tensor_add(A, A, E)
            # Interleave the diagonal (sum_i s*<I_c[i], T_c[i]>) into DVE slack
            # mid-loop: TD shard DMA after a few tiles, one kc-chunk of the
            # elementwise product every few text tiles.
            if t == 1:
                for kc in range(KC):
                    nc.sync.dma_start(out=TD[:, kc, :], in_=td_d[:, kc, :])
            if t >= 4 and t % 4 == 0 and t // 4 <= KC:
                kc = t // 4 - 1
                nc.vector.scalar_tensor_tensor(
                    out=dprod[:, kc, :], in0=IT[:, kc, :], scalar=1.0,
                    in1=TD[:, kc, :],
                    op0=ALU.mult, op1=ALU.mult,
                    accum_out=dacc8[:, kc:kc + 1],
                )

        # ---- diag chunk totals -> one per-partition scalar ----
        dacc = singles.tile([128, 1], F32)
        nc.vector.tensor_reduce(
            out=dacc, in_=dacc8, op=ALU.add, axis=mybir.AxisListType.X,
        )

        # ---- i2t row sums: partition-reduce A via ones-matmul, then ln ----
        psum_r = tailp.tile([1, LOCAL], F32, tag="tail")
        for h in range(2):
            nc.tensor.matmul(
                psum_r[:, h * 512:(h + 1) * 512], ones, A[:, h * 512:(h + 1) * 512],
                start=True, stop=True,
            )
        lse_r = singles.tile([1, LOCAL], F32)
        s_row = singles.tile([1, 1], F32)
        nc.scalar.activation(lse_r, psum_r, AF.Ln, accum_out=s_row)

        # ---- diag total: partition-reduce dacc ----
        psum_d = tailp.tile([1, LOCAL], F32, tag="tail")
        nc.tensor.matmul(psum_d[:, 0:1], ones, dacc, start=True, stop=True)

        # ---- AllReduce: [p, 0:64] t2i partials, [0, 64] s_row, [0, 65] diag ----
        arin = singles.tile([128, 66], F32)
        nc.vector.memset(arin, 0.0)
        nc.vector.tensor_copy(arin[:, 0:TT], colacc)
        nc.vector.tensor_copy(arin[0:1, TT:TT + 1], s_row)
        nc.vector.tensor_copy(arin[0:1, TT + 1:TT + 2], psum_d[0:1, 0:1])

        cc_in = drp.tile([128, 66], F32)
        cc_out = drp.tile([128, 66], F32, addr_space="Shared")
        nc.sync.dma_start(out=cc_in, in_=arin)
        nc.gpsimd.collective_compute(
            "AllReduce",
            ALU.add,
            replica_groups=[list(range(CORES))],
            ins=[cc_in[:]],
            outs=[cc_out[:]],
        )
        arout = singles.tile([128, 66], F32)
        nc.sync.dma_start(out=arout, in_=cc_out)

        # ---- t2i logsumexp over the reduced column sums ----
        lse_c = singles.tile([128, TT], F32)
        cl = singles.tile([128, 1], F32)
        nc.scalar.activation(lse_c, arout[:, 0:TT], AF.Ln, accum_out=cl)
        psum_s = tailp.tile([1, LOCAL], F32, tag="tail")
        nc.tensor.matmul(psum_s[:, 0:1], ones, cl, start=True, stop=True)

        # ---- loss = (S_col + S_row - 2*diag_tot)/(2N) + C ----
        f0 = singles.tile([1, 1], F32)
        nc.vector.tensor_add(f0, psum_s[0:1, 0:1], arout[0:1, TT:TT + 1])
        f1 = singles.tile([1, 1], F32)
        nc.vector.scalar_tensor_tensor(
            out=f1, in0=arout[0:1, TT + 1:TT + 2], scalar=-2.0, in1=f0,
            op0=ALU.mult, op1=ALU.add,
        )
        res = singles.tile([1, 1], F32)
        nc.scalar.activation(res, f1, AF.Copy, bias=C_SHIFT, scale=1.0 / (2.0 * N))
        nc.sync.dma_start(out=out_d[:], in_=res)

    nc.finalize()
    return nc


def _prep_inputs(image_features, text_features, logit_scale):
    s = float(np.asarray(logit_scale, dtype=np.float32))
    I = np.asarray(image_features, dtype=np.float32)
    T = np.asarray(text_features, dtype=np.float32)
    bf16 = ml_dtypes.bfloat16

    # tt[t, p, kc, c] = T[t*128 + c, kc*128 + p]
    tt = np.ascontiguousarray(
        T.reshape(TT, 128, KC, 128).transpose(0, 3, 2, 1)
    ).astype(bf16)

    in_maps = []
    for c in range(CORES):
        Ic = I[c * LOCAL:(c + 1) * LOCAL] * s
        it = np.ascontiguousarray(
            Ic.reshape(LOCAL, KC, 128).transpose(2, 1, 0)
        ).astype(bf16)
        Tc = T[c * LOCAL:(c + 1) * LOCAL]
        td = np.ascontiguousarray(
            Tc.reshape(LOCAL, KC, 128).transpose(2, 1, 0)
        ).astype(bf16)
        in_maps.append({"it": it, "tt": tt, "td": td})
    return in_maps


def _get_nc():
    if "nc" not in _CACHE:
        _CACHE["nc"] = _build_bass()
    return _CACHE["nc"]


def kernel(image_features, text_features, logit_scale, _trace=False):
    nc = _get_nc()
    in_maps = _prep_inputs(image_features, text_features, logit_scale)
    out = run_bass_kernel_spmd(nc, in_maps, list(range(CORES)), trace=_trace)
    loss = np.float32(out.results[0]["out"][0, 0])
    if _trace:
        return loss, out
    return loss
